# revision 1
# baseline (speedup 1.0000x reference)
"""Deformable Conv2d (DCNv2-style) Trainium2 Bass kernel.

Shards over 8 NeuronCores: core = b * 2 + ph  (b = batch 0..3, ph = pixel half).
Each core computes output pixels [ph*2048, (ph+1)*2048) of batch b.

Device pipeline per core:
  1. offset/mask 3x3 conv as 9 shifted-tap matmuls (PE, f32, PSUM accumulate)
  2. conv output PE-transposed to pixel-major; bilinear coords/coeffs on
     DVE/ACT (floor via RNE int cast of x-0.5); a second mod-16-ordered
     pipeline computes gather row indices so the dma_gather idx wrap
     ([16, n/16] partition layout) is a plain PE transpose
  3. U^T[pix, o] = x^T W_k per kernel tap k (PE, f32), staged to DRAM as f16
  4. dma_gather row-pairs (512B descriptors) of U^T at data-dependent rows
  5. per-pixel coeff scaling (tensor_scalar / ACT scale, per-partition
     scalars) and accumulation over (k, corner) via identity-matmuls in PSUM
  6. store pixel-major [2048, 128] f32; host reassembles + bias.
"""
import sys

sys.path.insert(0, "/opt/trn_rl_repo")

import numpy as np

import concourse.mybir as mybir
from concourse.ap import AP
from concourse.bacc import Bacc
from concourse.tile import TileContext
from concourse import bass_utils

F32 = mybir.dt.float32
F16 = mybir.dt.float16
I32 = mybir.dt.int32
I16 = mybir.dt.int16
Alu = mybir.AluOpType
Act = mybir.ActivationFunctionType

B, C, H, W = 4, 128, 64, 64
O, K, KK = 128, 3, 9
HWp = H * W
HALF = HWp // 2              # 2048 pixels per core
HROWS = 32
XR = 38                      # local halo rows: global [h0-3, h0+35); |dy|<2 safe
XPIX = XR * W                # 2560
NT = XPIX // 128             # 20 U tiles
UR = XPIX + 2                # U staging rows (zero rows 0 and UR-1)
G16 = HALF // 128            # 16 pixel groups


def _colsA(tile, row0, nrow=9):
    """A-pipeline view: [128, 16g x nrow] cols of cP (stride 27)."""
    t = tile[:, :]
    return AP(tensor=t.tensor, offset=t.offset + row0,
              ap=[t.ap[0], [27, G16], [1, nrow]])


def _colsB(tile, row0, nrow=9):
    t = tile[:, :]
    return AP(tensor=t.tensor, offset=t.offset + row0,
              ap=[t.ap[0], [18, G16], [1, nrow]])


def _build(nc, debug=False, stop_after=None):
    xh = nc.dram_tensor("xh", [C, XPIX], F16, kind="ExternalInput")
    wc = nc.dram_tensor("wc", [C, 9 * 27], F16, kind="ExternalInput")
    bvec = nc.dram_tensor("bvec", [27, 1], F32, kind="ExternalInput")
    wkt = nc.dram_tensor("wkt", [C, KK * O], F16, kind="ExternalInput")
    gyA = nc.dram_tensor("gyA", [128, 144], F32, kind="ExternalInput")
    gxA = nc.dram_tensor("gxA", [128, 144], F32, kind="ExternalInput")
    gyB = nc.dram_tensor("gyB", [128, 144], F32, kind="ExternalInput")
    gxB = nc.dram_tensor("gxB", [128, 144], F32, kind="ExternalInput")
    shiftv = nc.dram_tensor("shiftv", [128, 1], F32, kind="ExternalInput")
    ident = nc.dram_tensor("ident", [128, 128], F16, kind="ExternalInput")
    identf = nc.dram_tensor("identf", [128, 128], F32, kind="ExternalInput")
    out = nc.dram_tensor("out", [HALF, O], F32, kind="ExternalOutput")

    ustage = nc.dram_tensor("ustage", [KK * UR * 128], F16, kind="Internal")
    idxstage = nc.dram_tensor("idxstage", [16 * 18 * 128], I16, kind="Internal")

    if debug:
        dbg_conv = nc.dram_tensor("dbg_conv", [27, HALF], F32, kind="ExternalOutput")
        dbg_cT = nc.dram_tensor("dbg_cT", [128, G16 * 36], F32, kind="ExternalOutput")
        dbg_wrap = nc.dram_tensor("dbg_wrap", [16, 18 * 128], I16, kind="ExternalOutput")

    with TileContext(nc) as tc:
        with (
            tc.tile_pool(name="big", bufs=1) as big,
            tc.tile_pool(name="small", bufs=1) as small,
        ):
            x_sb = big.tile([C, XPIX], F16, tag="x_sb")
            nc.gpsimd.dma_start(x_sb[:, :], xh[:, :])
            wc_sb = small.tile([C, 9 * 27], F16, tag="wc")
            nc.scalar.dma_start(wc_sb[:, :], wc[:, :])
            bvec_sb = small.tile([27, 1], F32, tag="bvec")
            nc.scalar.dma_start(bvec_sb[:, :], bvec[:, :])
            wk_sb = big.tile([C, KK * O], F16, tag="wk")
            nc.sync.dma_start(wk_sb[:, :], wkt[:, :])
            gyA_sb = small.tile([128, 144], F32, tag="gyA")
            nc.scalar.dma_start(gyA_sb[:, :], gyA[:, :])
            gxA_sb = small.tile([128, 144], F32, tag="gxA")
            nc.scalar.dma_start(gxA_sb[:, :], gxA[:, :])
            gyB_sb = small.tile([128, 144], F32, tag="gyB")
            nc.scalar.dma_start(gyB_sb[:, :], gyB[:, :])
            gxB_sb = small.tile([128, 144], F32, tag="gxB")
            nc.scalar.dma_start(gxB_sb[:, :], gxB[:, :])
            shift_sb = small.tile([128, 1], F32, tag="shiftv")
            nc.scalar.dma_start(shift_sb[:, :], shiftv[:, :])
            id_sb = small.tile([128, 128], F16, tag="ident")
            nc.scalar.dma_start(id_sb[:, :], ident[:, :])
            idf_sb = small.tile([128, 128], F32, tag="identf")
            nc.scalar.dma_start(idf_sb[:, :], identf[:, :])

            # padded conv input: local rows 2..36 -> [C, 34*66], zero borders
            xpad = big.tile([C, 34 * 66], F16, tag="xpad")
            nc.gpsimd.memset(xpad[:, :], 0.0)
            nc.vector.tensor_copy(
                AP(tensor=xpad.tensor, offset=xpad[:, :].offset + 1,
                   ap=[xpad[:, :].ap[0], [66, 34], [1, W]]),
                AP(tensor=x_sb.tensor, offset=x_sb[:, :].offset + 2 * W,
                   ap=[x_sb[:, :].ap[0], [W, 34], [1, W]]),
            )

            # ---------- offset/mask conv ----------
            run_conv = stop_after != "loads"
            convR = big.tile([27, HALF], F32, tag="convR")
            pfront = tc.tile_pool(name="pfront", bufs=2, space="PSUM")
            psc = pfront.__enter__()

            def emit_conv_chunk(ch):
                pc = psc.tile([27, 512], F32, tag="pf")
                for th in range(3):
                    for tw in range(3):
                        tap = th * 3 + tw
                        rhs = AP(
                            tensor=xpad.tensor,
                            offset=xpad[:, :].offset + (ch * 8 + th) * 66 + tw,
                            ap=[xpad[:, :].ap[0], [66, 8], [1, W]],
                        )
                        nc.tensor.matmul(
                            pc[:, :], wc_sb[:, tap * 27:(tap + 1) * 27], rhs,
                            start=(tap == 0), stop=(tap == 8),
                        )
                nc.scalar.activation(
                    convR[0:27, ch * 512:(ch + 1) * 512], pc[:, :], Act.Identity,
                    bias=bvec_sb[:, 0:1], scale=1.0,
                )

            trunc = ("loads", "conv", "trans", "prologue")
            run_u = stop_after not in trunc
            run_gather = stop_after not in trunc + ("u",)
            # ---------- U^T matmuls + staging ----------
            zrow = None
            if run_u:
                zrow = small.tile([1, 128], F16, tag="zrow")
            if run_u:
                nc.vector.memset(zrow[:, :], 0.0)
            for uoff in ((0, (UR - 1) * 128) if run_u else ()):
                nc.scalar.dma_start(
                    AP(tensor=ustage, offset=uoff,
                       ap=[[UR * 128, KK], [1, 128]]),
                    AP(tensor=zrow.tensor, offset=zrow[:, :].offset,
                       ap=[zrow[:, :].ap[0], [0, KK], [1, 128]]))
            with (
                tc.tile_pool(name="psu", bufs=2, space="PSUM") as psu,
                tc.tile_pool(name="usb", bufs=1) as usbp,
            ):
                u16s = []
                for t in range(NT if run_u else 0):
                    pu = psu.tile([128, KK * O], F32, tag="pu")
                    lhsT = x_sb[:, t * 128:(t + 1) * 128]
                    nc.tensor.matmul(pu[:, 0:512], lhsT, wk_sb[:, 0:512],
                                     start=True, stop=False)
                    nc.tensor.matmul(pu[:, 512:1024], lhsT, wk_sb[:, 512:1024],
                                     start=True, stop=False)
                    nc.tensor.matmul(pu[:, 1024:1152], lhsT, wk_sb[:, 1024:1152],
                                     start=True, stop=True)
                    u16 = usbp.tile([128, KK * O], F16, tag=f"u16_{t}")
                    nc.scalar.activation(u16[:, :], pu[:, :], Act.Copy)
                    u16s.append(u16)
                    eng = nc.sync
                    eng.dma_start(
                        AP(tensor=ustage, offset=(1 + t * 128) * 128,
                           ap=[[128, 128], [UR * 128, KK], [1, 128]]),
                        u16[:, :].rearrange("p (k e) -> p k e", k=KK))
                    if run_conv and t % 2 == 1 and t // 2 < 4:
                        emit_conv_chunk(t // 2)
                if not run_u and run_conv:
                    for ch in range(4):
                        emit_conv_chunk(ch)
                if debug:
                    nc.sync.dma_start(dbg_conv[:, :], convR[:, :])

            # B-order conv copy: col P*16+g -> convB[:, g*128+P]
            run_trans = stop_after not in ("loads", "conv")
            convB = cB = None
            if run_trans:
              convB = big.tile([18, HALF], F32, tag="convB")
              cB = convB[:, :]
              nc.vector.tensor_copy(
                AP(tensor=cB.tensor, offset=cB.offset,
                   ap=[cB.ap[0], [1, HALF]]),
                AP(tensor=convR.tensor, offset=convR[:, :].offset,
                   ap=[[convR[:, :].ap[0][0], 18], [1, G16], [16, 128]]),
            )

            # transposes to pixel-major
            cP = cPB = None
            if run_trans:
                cP = big.tile([128, G16 * 27], F32, tag="cP")
                cPB = big.tile([128, G16 * 18], F32, tag="cPB")
            if True:
                for g in range(G16 if run_trans else 0):
                    ptb = psc.tile([128, 18], F32, tag="pf")
                    nc.tensor.transpose(
                        ptb[:, :], convB[:, g * 128:(g + 1) * 128], idf_sb[0:18, 0:18])
                    nc.vector.tensor_copy(cPB[:, g * 18:(g + 1) * 18], ptb[:, :])

            # ---------- A pipeline: coefficients (pixel-major, slot=pixel) ----------
            run_coef = stop_after not in ("loads", "conv", "trans")
            NSL = 20
            cw = itmp = cT = None
            if run_coef:
                cw = big.tile([128, NSL * 144], F32, tag="cw")
                itmp = small.tile([128, 144], I32, tag="itmp")
                itmp2 = small.tile([128, 144], I32, tag="itmp2")
                cT = big.tile([128, G16 * 36], F32, tag="cT")

            def S(q):
                return cw[:, q * 144:(q + 1) * 144]

            class _SkipCtx:
                pass
            def emit_A():
                with tc.tile_pool(name="psta", bufs=2, space="PSUM") as psta:
                    for g in range(G16):
                        pt = psta.tile([128, 27], F32, tag="ptA")
                        nc.tensor.transpose(
                            pt[:, :], convR[:, g * 128:(g + 1) * 128],
                            idf_sb[0:27, 0:27])
                        nc.scalar.activation(cP[:, g * 27:(g + 1) * 27], pt[:, :], Act.Copy)
                PY, PX, M, Y0, X0, FY, FX, Y1, X1 = range(9)
                CY0, CY1, VY0, VY1, VX0, VX1, IXC, T1, T2, T3 = range(9, 19)
                nc.vector.tensor_tensor(S(PY), _colsA(cP, 0), gyA_sb[:, :], Alu.add)
                nc.vector.tensor_tensor(S(PX), _colsA(cP, 9), gxA_sb[:, :], Alu.add)
                nc.scalar.activation(S(M), _colsA(cP, 18), Act.Sigmoid)
                # floors
                nc.vector.tensor_scalar(S(T1), S(PY), -0.5, None, Alu.add)
                nc.vector.tensor_copy(itmp[:, :], S(T1))
                nc.vector.tensor_copy(S(Y0), itmp[:, :])
                nc.vector.tensor_scalar(S(T1), S(PX), -0.5, None, Alu.add)
                nc.vector.tensor_copy(itmp[:, :], S(T1))
                nc.vector.tensor_copy(S(X0), itmp[:, :])
                nc.vector.tensor_tensor(S(FY), S(PY), S(Y0), Alu.subtract)
                nc.vector.tensor_tensor(S(FX), S(PX), S(X0), Alu.subtract)
                nc.vector.tensor_scalar(S(Y1), S(Y0), 1.0, None, Alu.add)
                nc.vector.tensor_scalar(S(X1), S(X0), 1.0, None, Alu.add)
                # validity
                nc.vector.tensor_scalar(S(CY0), S(Y0), 0.0, 63.0, Alu.max, Alu.min)
                nc.vector.tensor_tensor(S(VY0), S(CY0), S(Y0), Alu.is_equal)
                nc.vector.tensor_scalar(S(CY1), S(Y1), 0.0, 63.0, Alu.max, Alu.min)
                nc.vector.tensor_tensor(S(VY1), S(CY1), S(Y1), Alu.is_equal)
                nc.vector.tensor_scalar(S(T1), S(X0), 0.0, 63.0, Alu.max, Alu.min)
                nc.vector.tensor_tensor(S(VX0), S(T1), S(X0), Alu.is_equal)
                nc.vector.tensor_scalar(S(T1), S(X1), 0.0, 63.0, Alu.max, Alu.min)
                nc.vector.tensor_tensor(S(VX1), S(T1), S(X1), Alu.is_equal)
                # weights: wy0=(1-fy)*m*vy0 ; wy1=fy*m*vy1 ; ax0=(1-fx)*vx0 ; ax1=fx*vx1
                nc.vector.tensor_scalar(S(T1), S(FY), -1.0, 1.0, Alu.mult, Alu.add)
                nc.vector.tensor_tensor(S(T1), S(T1), S(M), Alu.mult)
                nc.vector.tensor_tensor(S(T1), S(T1), S(VY0), Alu.mult)     # wy0
                nc.vector.tensor_tensor(S(T2), S(FY), S(M), Alu.mult)
                nc.vector.tensor_tensor(S(T2), S(T2), S(VY1), Alu.mult)     # wy1
                nc.vector.tensor_scalar(S(T3), S(FX), -1.0, 1.0, Alu.mult, Alu.add)
                nc.vector.tensor_tensor(S(T3), S(T3), S(VX0), Alu.mult)     # ax0
                nc.vector.tensor_tensor(S(FX), S(FX), S(VX1), Alu.mult)     # ax1 (overwr)

                def cT_view(corner):
                    t = cT[:, :]
                    return AP(tensor=t.tensor, offset=t.offset + corner * 9,
                              ap=[t.ap[0], [36, G16], [1, 9]])

                nc.vector.tensor_tensor(cT_view(0), S(T1), S(T3), Alu.mult)  # c00
                nc.vector.tensor_tensor(cT_view(1), S(T1), S(FX), Alu.mult)  # c01
                nc.vector.tensor_tensor(cT_view(2), S(T2), S(T3), Alu.mult)  # c10
                nc.vector.tensor_tensor(cT_view(3), S(T2), S(FX), Alu.mult)  # c11
                if debug:
                    nc.sync.dma_start(dbg_cT[:, :], cT[:, :])


            if run_coef:
                # ---------- B pipeline: gather indices (slot P*16+g order) ----------
                bw = big.tile([128, 8 * 144], F32, tag="bw")
                idxPM = big.tile([128, 288], F32, tag="idxPM")

                def Sb(q):
                    return bw[:, q * 144:(q + 1) * 144]

                BPY, BPX, BY0, BX0, BT, BIX, BCY, BT2 = range(8)
                nc.vector.tensor_tensor(Sb(BPY), _colsB(cPB, 0), gyB_sb[:, :], Alu.add)
                nc.vector.tensor_tensor(Sb(BPX), _colsB(cPB, 9), gxB_sb[:, :], Alu.add)
                nc.vector.tensor_scalar(Sb(BT), Sb(BPY), -0.5, None, Alu.add)
                nc.vector.tensor_copy(itmp[:, :], Sb(BT))
                nc.vector.tensor_copy(Sb(BY0), itmp[:, :])
                nc.vector.tensor_scalar(Sb(BT), Sb(BPX), -0.5, None, Alu.add)
                nc.vector.tensor_copy(itmp[:, :], Sb(BT))
                nc.vector.tensor_copy(Sb(BX0), itmp[:, :])
                nc.vector.tensor_scalar(Sb(BIX), Sb(BX0), -1.0, 63.0, Alu.max, Alu.min)

                def idx_view(pair):
                    t = idxPM[:, :]
                    return AP(tensor=t.tensor, offset=t.offset + pair * 144,
                              ap=[t.ap[0], [1, G16], [16, 9]])

                # idx0 = clamp(y0)*64 + shift + ix
                nc.vector.tensor_scalar(Sb(BCY), Sb(BY0), 0.0, 63.0, Alu.max, Alu.min)
                nc.vector.tensor_scalar(Sb(BT2), Sb(BCY), 64.0, shift_sb[:, 0:1],
                                        Alu.mult, Alu.add)
                nc.vector.tensor_tensor(Sb(BT2), Sb(BT2), Sb(BIX), Alu.add)
                nc.vector.tensor_scalar(idx_view(0), Sb(BT2), 0.0, float(UR - 2),
                                        Alu.max, Alu.min)
                # idx1 = clamp(y0+1)*64 + shift + ix
                nc.vector.tensor_scalar(Sb(BCY), Sb(BY0), 1.0, None, Alu.add)
                nc.vector.tensor_scalar(Sb(BCY), Sb(BCY), 0.0, 63.0, Alu.max, Alu.min)
                nc.vector.tensor_scalar(Sb(BT2), Sb(BCY), 64.0, shift_sb[:, 0:1],
                                        Alu.mult, Alu.add)
                nc.vector.tensor_tensor(Sb(BT2), Sb(BT2), Sb(BIX), Alu.add)
                nc.vector.tensor_scalar(idx_view(1), Sb(BT2), 0.0, float(UR - 2),
                                        Alu.max, Alu.min)

                # idx transposes -> wrap rows [16, 128] each, cast to i16
                wrapS = big.tile([16, 18 * 128], I16, tag="wrapS")
                pfront.__exit__(None, None, None)
                with tc.tile_pool(name="psi", bufs=4, space="PSUM") as psi:
                    for pair in range(2):
                        for k in range(KK):
                            pw = psi.tile([16, 128], F32, tag="pw")
                            nc.tensor.transpose(
                                pw[:, :],
                                idxPM[:, pair * 144 + k * 16: pair * 144 + (k + 1) * 16],
                                idf_sb[:, :])
                            r = k * 2 + pair
                            nc.vector.tensor_copy(
                                wrapS[:, r * 128:(r + 1) * 128], pw[:, :])
                if debug:
                    nc.sync.dma_start(dbg_wrap[:, :], wrapS[:, :])
                # bounce to DRAM and back replicated x8
                nc.scalar.dma_start(
                    AP(tensor=idxstage, offset=0, ap=[[2304, 16], [1, 2304]]),
                    wrapS[:, :])
                idxW = big.tile([128, 18 * 128], I16, tag="idxW")
                for a in range(2):
                    nc.scalar.dma_start(
                        idxW[a * 64:(a + 1) * 64, :],
                        AP(tensor=idxstage, offset=0,
                           ap=[[0, 4], [2304, 16], [1, 2304]]))

            if not run_u:
                mark = {"loads": xpad, "conv": convR, "trans": cP,
                        "prologue": cT}[stop_after]
                nc.sync.dma_start(out[0:mark.shape[0], 0:64],
                                  mark[0:mark.shape[0], 0:64])
            if run_coef:
                emit_A()
            if run_u and not run_gather:
                nc.sync.dma_start(out[0:128, :], cT[:, 0:128])
            # ---------- gathers + combine ----------
            skip_combine = stop_after == "gather"
            with (
                tc.tile_pool(name="gat", bufs=3) as gat,
                tc.tile_pool(name="tmul", bufs=6) as tmul,
                tc.tile_pool(name="pso", bufs=1, space="PSUM") as pso,
                tc.tile_pool(name="osb", bufs=1) as osbp,
            ):
                po = None
                if not skip_combine:
                    po = pso.tile([128, HALF], F32, tag="po")
                for k in range(KK if run_gather else 0):
                    src_ap = AP(tensor=ustage, offset=k * UR * 128,
                                ap=[[128, UR - 1], [1, 256]])
                    gt = gat.tile([128, 2 * G16, 256], F16, tag="gt")
                    nc.gpsimd.dma_gather(
                        gt[:, :, :], src_ap,
                        idxW[:, k * 256:(k + 1) * 256],
                        2 * HALF, 2 * HALF, 256, elem_step=128,
                        single_packet=False)
                    if skip_combine:
                        sink = osbp.tile([128, 16], F16, tag="sink")
                        nc.vector.tensor_copy(sink[:, :], gt[:, 0, 0:16])
                        continue
                    for pair in range(2):
                        for half in range(2):
                            corner = pair * 2 + half
                            tb = tmul.tile([128, HALF], F16, tag="tb")
                            for g in range(G16):
                                sc = cT[:, g * 36 + corner * 9 + k:
                                        g * 36 + corner * 9 + k + 1]
                                dst = tb[:, g * 128:(g + 1) * 128]
                                srcg = gt[:, pair * G16 + g,
                                          half * 128:(half + 1) * 128]
                                if g % 4 != 3:
                                    nc.vector.tensor_scalar(dst, srcg, sc, None,
                                                            Alu.mult)
                                else:
                                    nc.scalar.activation(dst, srcg, Act.Copy,
                                                         scale=sc)
                            first = (k == 0 and pair == 0 and half == 0)
                            last = (k == KK - 1 and pair == 1 and half == 1)
                            for q in range(4):
                                nc.tensor.matmul(
                                    po[:, q * 512:(q + 1) * 512], id_sb[:, :],
                                    tb[:, q * 512:(q + 1) * 512],
                                    start=first, stop=last)
                if not skip_combine and run_gather:
                    osb = osbp.tile([128, HALF], F32, tag="osb")
                    for q in range(4):
                        sl = slice(q * 512, (q + 1) * 512)
                        nc.vector.tensor_copy(osb[:, sl], po[:, sl])
                        nc.sync.dma_start(
                            AP(tensor=out, offset=q * 4 * 128 * 128,
                               ap=[[128, 128], [128 * 128, 4], [1, 128]]),
                            osb[:, sl].rearrange("p (g e) -> p g e", g=4))

    nc.compile()
    return nc


_CACHE = {}


def _get_nc(debug=False, stop_after=None):
    key = (bool(debug), stop_after)
    if key not in _CACHE:
        nc = Bacc()
        _CACHE[key] = _build(nc, debug=debug, stop_after=stop_after)
    return _CACHE[key]


def _grid_tables(h0, order):
    """[128, 144] tables: [P, g*9+k] = gy/gx of (pixel, k) for the given
    slot->pixel order: 'A': pixel = g*128+P ; 'B': pixel = P*16+g."""
    ki = (np.arange(KK) // 3).astype(np.float32)
    kj = (np.arange(KK) % 3).astype(np.float32)
    P = np.arange(128)
    g = np.arange(G16)
    if order == "A":
        pix = g[None, :] * 128 + P[:, None]          # [128, 16]
    else:
        pix = P[:, None] * 16 + g[None, :]
    gy = (h0 + pix // W)[:, :, None] + (ki - 1.0)[None, None, :]
    gx = (pix % W)[:, :, None] + (kj - 1.0)[None, None, :]
    return (np.ascontiguousarray(gy.reshape(128, 144).astype(np.float32)),
            np.ascontiguousarray(gx.reshape(128, 144).astype(np.float32)))


def _prep_inputs(x, w_off, b_off, w_mask, b_mask, weight, bias):
    x = np.asarray(x, np.float32)
    w_off = np.asarray(w_off, np.float32)
    b_off = np.asarray(b_off, np.float32)
    w_mask = np.asarray(w_mask, np.float32)
    b_mask = np.asarray(b_mask, np.float32)
    weight = np.asarray(weight, np.float32)

    w_cat = np.concatenate([w_off[0::2], w_off[1::2], w_mask], axis=0)
    b_cat = np.concatenate([b_off[0::2], b_off[1::2], b_mask])
    wc = np.ascontiguousarray(
        w_cat.reshape(27, C, 9).transpose(1, 2, 0).reshape(C, 9 * 27)).astype(np.float16)
    bvec = np.ascontiguousarray(b_cat.reshape(27, 1))
    wkt = np.ascontiguousarray(
        weight.reshape(O, C, KK).transpose(1, 2, 0).reshape(C, KK * O)).astype(np.float16)
    ident = np.eye(128, dtype=np.float16)
    identf = np.eye(128, dtype=np.float32)

    in_maps = []
    for core in range(8):
        b = core // 2
        ph = core % 2
        h0 = ph * HROWS
        hl = h0 - 3
        xb = x[b].reshape(C, H, W)
        xhh = np.zeros((C, XR, W), np.float32)
        for r in range(XR):
            gr = hl + r
            if 0 <= gr < H:
                xhh[:, r] = xb[:, gr]
        gyA, gxA = _grid_tables(h0, "A")
        gyB, gxB = _grid_tables(h0, "B")
        shiftv = np.full((128, 1), 1.0 - hl * 64.0, np.float32)
        in_maps.append({
            "xh": np.ascontiguousarray(xhh.reshape(C, XPIX)).astype(np.float16),
            "wc": wc, "bvec": bvec, "wkt": wkt,
            "gyA": gyA, "gxA": gxA, "gyB": gyB, "gxB": gxB,
            "shiftv": shiftv, "ident": ident, "identf": identf,
        })
    return in_maps


def kernel(x, w_off, b_off, w_mask, b_mask, weight, bias, _debug=False, _trace=False):
    nc = _get_nc(debug=_debug)
    in_maps = _prep_inputs(x, w_off, b_off, w_mask, b_mask, weight, bias)
    res = bass_utils.run_bass_kernel_spmd(
        nc, in_maps, core_ids=list(range(8)), trace=_trace)
    out = np.zeros((B, O, H, W), np.float32)
    for core in range(8):
        b, ph = core // 2, core % 2
        chunk = res.results[core]["out"]
        out[b, :, ph * HROWS:(ph + 1) * HROWS, :] = (
            chunk.reshape(HROWS, W, O).transpose(2, 0, 1))
    out += np.asarray(bias, np.float32)[None, :, None, None]
    if _debug or _trace:
        kernel._last = res
    return out

